# revision 1
# baseline (speedup 1.0000x reference)
"""Trainium2 Bass kernel for the KnowledgeGraphEmbedding loss.

Computes, for P=1024 relations sharded 128-per-core across 8 NeuronCores:
    li = Lp_w[p] @ wi          (wi = tag_rep[tag1_idx])
    rj = Rp_w[p] @ wj          (wj = tag_rep[tag2_idx])
    dist[p] = sum_h (li - rj)^2
    out = [dist*rel, dist*(1-rel), rel, 1-rel]   (rel in {0,1})

Device strategy (memory-bound; ~92MB of weights streamed per core):
  - partition dim = relation (128 per core); K h-rows per tile iteration
  - tile layout [L-block | R-block], each fully contiguous per partition
  - DVE tensor_mul (in-place) by a broadcast [wi.. | -wj..] tile
  - per h: ScalarE activation(Copy) with accum_out reduces the (L,R) row
    pair in one pass -> diff[p, h] = li - rj
  - dist via one activation(Square, accum_out)
  - output bins via tensor_scalar ops on [128, 4]
"""

from contextlib import ExitStack

import numpy as np

N_CORES = 8
P_TOTAL = 1024
H = 300
E = 300
P_LOC = P_TOTAL // N_CORES  # 128 relations per core
K = 12                      # h-rows per tile iteration
N_ITER = H // K             # 25
KE = K * E

# Set by test harness to capture a profile; kernel() stores results here.
TRACE = False
LAST_RESULT = None

_CACHE: dict = {}


def _build_nc():
    import concourse.bacc as bacc
    import concourse.mybir as mybir
    import concourse.tile as tile

    f32 = mybir.dt.float32

    nc = bacc.Bacc("TRN2", debug=False)

    lw = nc.dram_tensor("lw", [P_LOC, H * E], f32, kind="ExternalInput").ap()
    rw = nc.dram_tensor("rw", [P_LOC, H * E], f32, kind="ExternalInput").ap()
    wv = nc.dram_tensor("wv", [P_LOC, 2 * E], f32, kind="ExternalInput").ap()
    rm = nc.dram_tensor("rm", [P_LOC, 2], f32, kind="ExternalInput").ap()
    out = nc.dram_tensor("out", [P_LOC, 4], f32, kind="ExternalOutput").ap()

    with tile.TileContext(nc) as tc, ExitStack() as ctx:
        const_pool = ctx.enter_context(tc.tile_pool(name="const", bufs=1))
        data_pool = ctx.enter_context(tc.tile_pool(name="data", bufs=4))

        wv_sb = const_pool.tile([P_LOC, 2 * E], f32)
        nc.sync.dma_start(wv_sb[:], wv[:])
        rm_sb = const_pool.tile([P_LOC, 2], f32)
        nc.sync.dma_start(rm_sb[:], rm[:])

        # wrep = [wi repeated K | -wj repeated K], matching the tile layout.
        wrep = const_pool.tile([P_LOC, 2 * KE], f32)
        for j in range(K):
            nc.vector.tensor_copy(wrep[:, j * E : (j + 1) * E], wv_sb[:, 0:E])
            nc.vector.tensor_copy(
                wrep[:, KE + j * E : KE + (j + 1) * E], wv_sb[:, E : 2 * E]
            )

        diff = const_pool.tile([P_LOC, H], f32)

        # Reduce-engine split: first K_DVE h-slots per tile reduce on the
        # (less loaded) vector engine, the rest on ScalarE. Balances
        # ACT ~1.06us/op against DVE's 195us of multiplies + ~0.5us/op.
        K_DVE = 3

        for t in range(N_ITER):
            dt_ = data_pool.tile([P_LOC, 2 * KE], f32)
            # L on the SP HWDGE ring, R on the ACT HWDGE ring: two physical
            # descriptor rings in parallel instead of one.
            nc.sync.dma_start(dt_[:, 0:KE], lw[:, t * KE : (t + 1) * KE])
            nc.scalar.dma_start(dt_[:, KE : 2 * KE], rw[:, t * KE : (t + 1) * KE])
            nc.vector.tensor_mul(dt_[:, 0:KE], dt_[:, 0:KE], wrep[:, 0:KE])
            nc.vector.tensor_mul(
                dt_[:, KE : 2 * KE], dt_[:, KE : 2 * KE], wrep[:, KE : 2 * KE]
            )
            dt_v = dt_.rearrange("p (s k e) -> p s k e", s=2, k=K)
            for j in range(K):
                # diff[:, h] = sum(L_row*wi) + sum(R_row*(-wj)) in one pass
                if j < K_DVE:
                    nc.vector.tensor_scalar(
                        out=dt_v[:, :, j, :],
                        in0=dt_v[:, :, j, :],
                        scalar1=1.0,
                        scalar2=0.0,
                        op0=mybir.AluOpType.mult,
                        op1=mybir.AluOpType.add,
                        accum_out=diff[:, t * K + j : t * K + j + 1],
                    )
                else:
                    nc.scalar.activation(
                        dt_v[:, :, j, :],
                        dt_v[:, :, j, :],
                        mybir.ActivationFunctionType.Copy,
                        accum_out=diff[:, t * K + j : t * K + j + 1],
                    )

        dist = const_pool.tile([P_LOC, 1], f32)
        sq = const_pool.tile([P_LOC, H], f32)
        nc.scalar.activation(
            sq[:], diff[:], mybir.ActivationFunctionType.Square, accum_out=dist[:]
        )

        out_sb = const_pool.tile([P_LOC, 4], f32)
        nc.vector.tensor_scalar_mul(out_sb[:, 0:2], rm_sb[:, 0:2], dist[:, 0:1])
        nc.vector.tensor_copy(out_sb[:, 2:4], rm_sb[:, 0:2])
        nc.sync.dma_start(out[:], out_sb[:])

    nc.compile()
    return nc


def kernel(tag_rep, Lp_w, Rp_w, relation, tag1_idx, tag2_idx):
    global LAST_RESULT
    from concourse.bass_utils import run_bass_kernel_spmd

    if "nc" not in _CACHE:
        _CACHE["nc"] = _build_nc()
    nc = _CACHE["nc"]

    tag_rep = np.asarray(tag_rep)
    Lp_w = np.asarray(Lp_w)
    Rp_w = np.asarray(Rp_w)
    rel = np.asarray(relation).astype(np.float32)  # values in {0, 1}

    wi = tag_rep[int(tag1_idx)].astype(np.float32)
    wj = tag_rep[int(tag2_idx)].astype(np.float32)
    wv_row = np.concatenate([wi, -wj])  # [600]
    wv = np.ascontiguousarray(np.broadcast_to(wv_row, (P_LOC, 2 * E)))

    in_maps = []
    for c in range(N_CORES):
        sl = slice(c * P_LOC, (c + 1) * P_LOC)
        rel_c = rel[sl]
        in_maps.append(
            {
                "lw": Lp_w[sl].reshape(P_LOC, H * E),
                "rw": Rp_w[sl].reshape(P_LOC, H * E),
                "wv": wv,
                "rm": np.ascontiguousarray(np.stack([rel_c, 1.0 - rel_c], axis=1)),
            }
        )

    kw = {}
    if TRACE:
        kw = dict(trace=True, trace_cores=[0])
    res = run_bass_kernel_spmd(nc, in_maps, core_ids=list(range(N_CORES)), **kw)
    LAST_RESULT = res

    out_full = np.empty((4, P_TOTAL), dtype=np.float32)
    for c in range(N_CORES):
        out_full[:, c * P_LOC : (c + 1) * P_LOC] = res.results[c]["out"].T
    return out_full



# revision 3
# speedup vs baseline: 1.4759x; 1.4759x over previous
"""Trainium2 Bass kernel for the KnowledgeGraphEmbedding loss.

Computes, for P=1024 relations sharded 128-per-core across 8 NeuronCores:
    li = Lp_w[p] @ wi          (wi = tag_rep[tag1_idx])
    rj = Rp_w[p] @ wj          (wj = tag_rep[tag2_idx])
    dist[p] = sum_h (li - rj)^2
    out = [dist*rel, dist*(1-rel), rel, 1-rel]   (rel in {0,1})

Device strategy (memory-bound):
  - weights are downcast to bf16 on the host and interleaved as
    [p, h, (L-row | R-row)] so each h owns a contiguous 600-elem row;
    bf16 halves HBM traffic vs f32: ~46MB/core at ~358GB/s -> ~129us
  - per tile: one big DVE tensor_mul by broadcast [wi | -wj] (bf16 2x mode)
  - per h-row reduce to diff[p,h]: first N_DVE rows via DVE tensor_scalar
    (4x mode) with accum_out, rest via ScalarE activation(Copy) accum
  - dist via one ScalarE activation(Square, accum_out)
  - output bins via tensor_scalar ops on [128, 4]
"""

from contextlib import ExitStack

import numpy as np
import ml_dtypes

BF16 = ml_dtypes.bfloat16

N_CORES = 8
P_TOTAL = 1024
H = 300
E = 300
ROW = 2 * E                 # interleaved [L-row | R-row]
P_LOC = P_TOTAL // N_CORES  # 128 relations per core
K = 20                      # h-rows per tile iteration
N_ITER = H // K             # 15
KR = K * ROW
N_DVE = 11                  # rows per tile reduced on DVE (rest on ACT)

# Set by test harness to capture a profile; kernel() stores results here.
TRACE = False
LAST_RESULT = None

_CACHE: dict = {}


def _build_nc():
    import concourse.bacc as bacc
    import concourse.mybir as mybir
    import concourse.tile as tile

    f32 = mybir.dt.float32
    bf16 = mybir.dt.bfloat16

    nc = bacc.Bacc("TRN2", debug=False)

    wlr = nc.dram_tensor("wlr", [P_LOC, H * ROW], bf16, kind="ExternalInput").ap()
    wv = nc.dram_tensor("wv", [P_LOC, ROW], bf16, kind="ExternalInput").ap()
    rm = nc.dram_tensor("rm", [P_LOC, 2], f32, kind="ExternalInput").ap()
    out = nc.dram_tensor("out", [P_LOC, 4], f32, kind="ExternalOutput").ap()

    with tile.TileContext(nc) as tc, ExitStack() as ctx:
        const_pool = ctx.enter_context(tc.tile_pool(name="const", bufs=1))
        data_pool = ctx.enter_context(tc.tile_pool(name="data", bufs=3))

        wv_sb = const_pool.tile([P_LOC, ROW], bf16)
        nc.sync.dma_start(wv_sb[:], wv[:])
        # wrep = [wi | -wj] repeated K times to cover a whole tile in one mul.
        wrep = const_pool.tile([P_LOC, KR], bf16)
        for j in range(K):
            nc.vector.tensor_copy(wrep[:, j * ROW : (j + 1) * ROW], wv_sb[:])
        rm_sb = const_pool.tile([P_LOC, 2], f32)
        nc.sync.dma_start(rm_sb[:], rm[:])

        diff = const_pool.tile([P_LOC, H], f32)

        for t in range(N_ITER):
            dt_ = data_pool.tile([P_LOC, KR], bf16)
            # Alternate the two physical HWDGE rings (SP / ACT).
            eng = nc.sync if t % 2 == 0 else nc.scalar
            eng.dma_start(dt_[:], wlr[:, t * KR : (t + 1) * KR])
            nc.vector.tensor_mul(dt_[:], dt_[:], wrep[:])
            dt_v = dt_.rearrange("p (k r) -> p k r", k=K)
            for j in range(K):
                acc = diff[:, t * K + j : t * K + j + 1]
                if j < N_DVE:
                    nc.vector.tensor_scalar(
                        out=dt_v[:, j, :],
                        in0=dt_v[:, j, :],
                        scalar1=1.0,
                        scalar2=0.0,
                        op0=mybir.AluOpType.mult,
                        op1=mybir.AluOpType.add,
                        accum_out=acc,
                    )
                else:
                    nc.scalar.activation(
                        dt_v[:, j, :],
                        dt_v[:, j, :],
                        mybir.ActivationFunctionType.Copy,
                        accum_out=acc,
                    )

        dist = const_pool.tile([P_LOC, 1], f32)
        sq = const_pool.tile([P_LOC, H], f32)
        nc.scalar.activation(
            sq[:], diff[:], mybir.ActivationFunctionType.Square, accum_out=dist[:]
        )

        out_sb = const_pool.tile([P_LOC, 4], f32)
        nc.vector.tensor_scalar_mul(out_sb[:, 0:2], rm_sb[:, 0:2], dist[:, 0:1])
        nc.vector.tensor_copy(out_sb[:, 2:4], rm_sb[:, 0:2])
        nc.sync.dma_start(out[:], out_sb[:])

    nc.compile()
    return nc


def kernel(tag_rep, Lp_w, Rp_w, relation, tag1_idx, tag2_idx):
    global LAST_RESULT
    from concourse.bass_utils import run_bass_kernel_spmd

    if "nc" not in _CACHE:
        _CACHE["nc"] = _build_nc()
    nc = _CACHE["nc"]

    tag_rep = np.asarray(tag_rep)
    Lp_w = np.asarray(Lp_w)
    Rp_w = np.asarray(Rp_w)
    rel = np.asarray(relation).astype(np.float32)  # values in {0, 1}

    wi = tag_rep[int(tag1_idx)].astype(np.float32)
    wj = tag_rep[int(tag2_idx)].astype(np.float32)
    wv_row = np.concatenate([wi, -wj]).astype(BF16)  # [600]
    wv = np.ascontiguousarray(np.broadcast_to(wv_row, (P_LOC, ROW)))

    # Interleave [p, h, (L | R)] and downcast to bf16 in one shot.
    big = np.empty((P_TOTAL, H, ROW), dtype=BF16)
    big[:, :, :E] = Lp_w
    big[:, :, E:] = Rp_w
    big = big.reshape(P_TOTAL, H * ROW)

    in_maps = []
    for c in range(N_CORES):
        sl = slice(c * P_LOC, (c + 1) * P_LOC)
        rel_c = rel[sl]
        in_maps.append(
            {
                "wlr": big[sl],
                "wv": wv,
                "rm": np.ascontiguousarray(np.stack([rel_c, 1.0 - rel_c], axis=1)),
            }
        )

    kw = {}
    if TRACE:
        kw = dict(trace=True, trace_cores=[0])
    res = run_bass_kernel_spmd(nc, in_maps, core_ids=list(range(N_CORES)), **kw)
    LAST_RESULT = res

    out_full = np.empty((4, P_TOTAL), dtype=np.float32)
    for c in range(N_CORES):
        out_full[:, c * P_LOC : (c + 1) * P_LOC] = res.results[c]["out"].T
    return out_full


# revision 4
# speedup vs baseline: 2.7142x; 1.8390x over previous
"""Trainium2 Bass kernel for the KnowledgeGraphEmbedding loss.

Computes, for P=1024 relations sharded 128-per-core across 8 NeuronCores:
    li = Lp_w[p] @ wi          (wi = tag_rep[tag1_idx])
    rj = Rp_w[p] @ wj          (wj = tag_rep[tag2_idx])
    dist[p] = sum_h (li - rj)^2
    out = [dist*rel, dist*(1-rel), rel, 1-rel]   (rel in {0,1})

Device strategy (memory-bound):
  - weights are downcast to bf16 on the host and interleaved as
    [p, h, (L-row | R-row)] so each h owns a contiguous 600-elem row;
    bf16 halves HBM traffic vs f32: ~46MB/core at ~358GB/s -> ~129us
  - per tile: one big DVE tensor_mul by broadcast [wi | -wj] (bf16 2x mode)
  - per h-row reduce to diff[p,h]: first N_DVE rows via DVE tensor_scalar
    (4x mode) with accum_out, rest via ScalarE activation(Copy) accum
  - dist via one ScalarE activation(Square, accum_out)
  - output bins via tensor_scalar ops on [128, 4]
"""

from contextlib import ExitStack

import numpy as np
import ml_dtypes

BF16 = ml_dtypes.bfloat16

N_CORES = 8
P_TOTAL = 1024
H = 300
E = 300
ROW = 2 * E                 # interleaved [L-row | R-row]
P_LOC = P_TOTAL // N_CORES  # 128 relations per core
K = 20                      # h-rows per tile iteration
N_ITER = H // K             # 15
KR = K * ROW
N_DVE = 2                   # rows per tile reduced on DVE (rest on ACT)
# Trace-measured: DVE tensor_scalar+accum runs 1x (~685ns/row), ACT
# activation+accum ~442ns/row; DVE also carries the 94us of muls, so
# balance puts ~2 rows/tile on DVE and 18 on ACT (~118us each engine).

# Set by test harness to capture a profile; kernel() stores results here.
TRACE = False
LAST_RESULT = None

_CACHE: dict = {}


def _build_nc():
    import concourse.bacc as bacc
    import concourse.mybir as mybir
    import concourse.tile as tile

    f32 = mybir.dt.float32
    bf16 = mybir.dt.bfloat16

    nc = bacc.Bacc("TRN2", debug=False)

    wlr = nc.dram_tensor("wlr", [P_LOC, H * ROW], bf16, kind="ExternalInput").ap()
    wv = nc.dram_tensor("wv", [P_LOC, ROW], bf16, kind="ExternalInput").ap()
    rm = nc.dram_tensor("rm", [P_LOC, 2], f32, kind="ExternalInput").ap()
    out = nc.dram_tensor("out", [P_LOC, 4], f32, kind="ExternalOutput").ap()

    with tile.TileContext(nc) as tc, ExitStack() as ctx:
        const_pool = ctx.enter_context(tc.tile_pool(name="const", bufs=1))
        data_pool = ctx.enter_context(tc.tile_pool(name="data", bufs=3))

        wv_sb = const_pool.tile([P_LOC, ROW], bf16)
        nc.sync.dma_start(wv_sb[:], wv[:])
        # wrep = [wi | -wj] repeated K times to cover a whole tile in one mul.
        wrep = const_pool.tile([P_LOC, KR], bf16)
        for j in range(K):
            nc.vector.tensor_copy(wrep[:, j * ROW : (j + 1) * ROW], wv_sb[:])
        rm_sb = const_pool.tile([P_LOC, 2], f32)
        nc.sync.dma_start(rm_sb[:], rm[:])

        diff = const_pool.tile([P_LOC, H], f32)

        for t in range(N_ITER):
            dt_ = data_pool.tile([P_LOC, KR], bf16)
            # Alternate the two physical HWDGE rings (SP / ACT).
            eng = nc.sync if t % 2 == 0 else nc.scalar
            eng.dma_start(dt_[:], wlr[:, t * KR : (t + 1) * KR])
            nc.vector.tensor_mul(dt_[:], dt_[:], wrep[:])
            dt_v = dt_.rearrange("p (k r) -> p k r", k=K)
            for j in range(K):
                acc = diff[:, t * K + j : t * K + j + 1]
                if j < N_DVE:
                    nc.vector.tensor_scalar(
                        out=dt_v[:, j, :],
                        in0=dt_v[:, j, :],
                        scalar1=1.0,
                        scalar2=0.0,
                        op0=mybir.AluOpType.mult,
                        op1=mybir.AluOpType.add,
                        accum_out=acc,
                    )
                else:
                    nc.scalar.activation(
                        dt_v[:, j, :],
                        dt_v[:, j, :],
                        mybir.ActivationFunctionType.Copy,
                        accum_out=acc,
                    )

        dist = const_pool.tile([P_LOC, 1], f32)
        sq = const_pool.tile([P_LOC, H], f32)
        nc.scalar.activation(
            sq[:], diff[:], mybir.ActivationFunctionType.Square, accum_out=dist[:]
        )

        out_sb = const_pool.tile([P_LOC, 4], f32)
        nc.vector.tensor_scalar_mul(out_sb[:, 0:2], rm_sb[:, 0:2], dist[:, 0:1])
        nc.vector.tensor_copy(out_sb[:, 2:4], rm_sb[:, 0:2])
        nc.sync.dma_start(out[:], out_sb[:])

    nc.compile()
    return nc


def kernel(tag_rep, Lp_w, Rp_w, relation, tag1_idx, tag2_idx):
    global LAST_RESULT
    from concourse.bass_utils import run_bass_kernel_spmd

    if "nc" not in _CACHE:
        _CACHE["nc"] = _build_nc()
    nc = _CACHE["nc"]

    tag_rep = np.asarray(tag_rep)
    Lp_w = np.asarray(Lp_w)
    Rp_w = np.asarray(Rp_w)
    rel = np.asarray(relation).astype(np.float32)  # values in {0, 1}

    wi = tag_rep[int(tag1_idx)].astype(np.float32)
    wj = tag_rep[int(tag2_idx)].astype(np.float32)
    wv_row = np.concatenate([wi, -wj]).astype(BF16)  # [600]
    wv = np.ascontiguousarray(np.broadcast_to(wv_row, (P_LOC, ROW)))

    # Interleave [p, h, (L | R)] and downcast to bf16 in one shot.
    big = np.empty((P_TOTAL, H, ROW), dtype=BF16)
    big[:, :, :E] = Lp_w
    big[:, :, E:] = Rp_w
    big = big.reshape(P_TOTAL, H * ROW)

    in_maps = []
    for c in range(N_CORES):
        sl = slice(c * P_LOC, (c + 1) * P_LOC)
        rel_c = rel[sl]
        in_maps.append(
            {
                "wlr": big[sl],
                "wv": wv,
                "rm": np.ascontiguousarray(np.stack([rel_c, 1.0 - rel_c], axis=1)),
            }
        )

    kw = {}
    if TRACE:
        kw = dict(trace=True, trace_cores=[0])
    res = run_bass_kernel_spmd(nc, in_maps, core_ids=list(range(N_CORES)), **kw)
    LAST_RESULT = res

    out_full = np.empty((4, P_TOTAL), dtype=np.float32)
    for c in range(N_CORES):
        out_full[:, c * P_LOC : (c + 1) * P_LOC] = res.results[c]["out"].T
    return out_full


# revision 10
# speedup vs baseline: 2.9149x; 1.0740x over previous
"""Trainium2 Bass kernel (fp8 TensorEngine path) for KnowledgeGraphEmbedding.

Per core (128 relations):
    diff[p,h] = [wi | -wj] . [L[p,h,:] | R[p,h,:]]   (600-dim contraction)
    dist[p]   = sum_h (diff/SCALE)^2
    out       = [dist*rel, dist*(1-rel), rel, 1-rel]

Strategy (memory-bound; fp8 quarters HBM traffic vs f32):
  - host: scale L,R by SCALE, quantize to fp8e4, transpose to
    contraction-major, pack for DoubleRow ([K,2,N] k-subtile pairs)
  - PE: per 512-column N-tile, 3 DoubleRow matmuls contract 600 dims
    (256+256+88) into psum [1, 512] f32
  - drains: DVE/ACT alternate copying psum -> staging rows [1, 12800]
  - reshape: 25-N-tile staging chunks DMA'd into diff [128, 300]
  - post: ACT Square(scale=1/SCALE) w/ accum -> dist; bins via DVE
"""

from contextlib import ExitStack

import numpy as np
import ml_dtypes

E4 = ml_dtypes.float8_e4m3

N_CORES = 8
P_TOTAL = 1024
H = 300
E = 300
ROW = 2 * E                 # merged [L-row | R-row] contraction dim
P_LOC = P_TOTAL // N_CORES  # 128
N = 512                     # psum bank columns per N-tile
NT = (P_LOC * H) // N       # 75 N-tiles per core
GRP = 5                     # N-tiles per DMA group
N_GRP = NT // GRP           # 15
CHUNKS = [(0, 128), (256, 128), (512, 44)]  # (row offset, k) DoubleRow pairs
SCALE = 32.0
STAGE_NT = 25               # N-tiles per staging chunk
STAGE_W = STAGE_NT * N      # 12800

TRACE = False
LAST_RESULT = None
USE_GPSIMD = False          # SWDGE ring for x2/reshape DMAs (crashes exec unit?)

_CACHE: dict = {}


def _stage_dmas(nc, diff, stage, k):
    """Emit DMAs moving staging chunk k ([1, 12800] f32, global columns
    [k*12800, (k+1)*12800) in p-major (p,h) order) into diff [128, 300]."""
    eng = nc.gpsimd if USE_GPSIMD else nc.sync
    g0 = k * STAGE_W
    pos = 0
    while pos < STAGE_W:
        g = g0 + pos
        p, h = divmod(g, H)
        if h != 0:
            run = min(H - h, STAGE_W - pos)
            eng.dma_start(diff[p : p + 1, h : h + run], stage[0:1, pos : pos + run])
            pos += run
            continue
        rows = (STAGE_W - pos) // H
        if rows == 0:
            run = STAGE_W - pos
            eng.dma_start(diff[p : p + 1, 0:run], stage[0:1, pos : pos + run])
            pos += run
        else:
            eng.dma_start(diff[p : p + rows, 0:H], stage[0:1, pos : pos + rows * H])
            pos += rows * H


def _build_nc():
    import concourse.bacc as bacc
    import concourse.mybir as mybir
    import concourse.tile as tile

    f32 = mybir.dt.float32
    fp8 = mybir.dt.float8e4

    nc = bacc.Bacc("TRN2", debug=False)

    x0 = nc.dram_tensor("x0", [128, NT * 1024], fp8, kind="ExternalInput").ap()
    x1 = nc.dram_tensor("x1", [128, NT * 1024], fp8, kind="ExternalInput").ap()
    x2 = nc.dram_tensor("x2", [44, NT * 1024], fp8, kind="ExternalInput").ap()
    wd = nc.dram_tensor("wd", [128, 96], fp8, kind="ExternalInput").ap()
    rm = nc.dram_tensor("rm", [P_LOC, 2], f32, kind="ExternalInput").ap()
    out = nc.dram_tensor("out", [P_LOC, 4], f32, kind="ExternalOutput").ap()

    with tile.TileContext(nc) as tc, ExitStack() as ctx:
        const_pool = ctx.enter_context(tc.tile_pool(name="const", bufs=1))
        data_pool = ctx.enter_context(tc.tile_pool(name="data", bufs=4))
        stage_pool = ctx.enter_context(tc.tile_pool(name="stage", bufs=2))
        psum_pool = ctx.enter_context(
            tc.tile_pool(name="ps", bufs=8, space=mybir.MemorySpace.PSUM)
        )

        wd_sb = const_pool.tile([128, 96], fp8)
        nc.sync.dma_start(wd_sb[:], wd[:])
        wd_v = wd_sb.rearrange("p (c s x) -> p c s x", c=3, s=2)  # x = 16
        rm_sb = const_pool.tile([P_LOC, 2], f32)
        nc.sync.dma_start(rm_sb[:], rm[:])

        diff = const_pool.tile([P_LOC, H], f32)

        stage = None
        for g in range(N_GRP):
            a0 = data_pool.tile([128, GRP * 1024], fp8)
            nc.sync.dma_start(a0[:], x0[:, g * GRP * 1024 : (g + 1) * GRP * 1024])
            a1 = data_pool.tile([128, GRP * 1024], fp8)
            nc.scalar.dma_start(a1[:], x1[:, g * GRP * 1024 : (g + 1) * GRP * 1024])
            a2 = data_pool.tile([44, GRP * 1024], fp8)
            eng2 = nc.gpsimd if USE_GPSIMD else (nc.scalar if g % 2 == 0 else nc.sync)
            eng2.dma_start(a2[:], x2[:, g * GRP * 1024 : (g + 1) * GRP * 1024])
            tiles = (a0, a1, a2)
            for i in range(GRP):
                m = g * GRP + i
                if m % STAGE_NT == 0:
                    stage = stage_pool.tile([1, STAGE_W], f32)
                ps = psum_pool.tile([1, N], f32)
                for c, (_, ksz) in enumerate(CHUNKS):
                    rhs = tiles[c][0:ksz, i * 1024 : (i + 1) * 1024].rearrange(
                        "p (s n) -> p s n", s=2
                    )
                    nc.tensor.matmul(
                        ps[:],
                        lhsT=wd_v[0:ksz, c, :, 0:1],
                        rhs=rhs,
                        start=(c == 0),
                        stop=(c == 2),
                        perf_mode=mybir.MatmulPerfMode.DoubleRow,
                    )
                dst = stage[0:1, (m % STAGE_NT) * N : (m % STAGE_NT + 1) * N]
                if m % 2 == 0:
                    nc.vector.tensor_copy(dst, ps[:])
                else:
                    nc.scalar.copy(dst, ps[:])
                if m % STAGE_NT == STAGE_NT - 1:
                    _stage_dmas(nc, diff, stage, m // STAGE_NT)

        dist = const_pool.tile([P_LOC, 1], f32)
        sq = const_pool.tile([P_LOC, H], f32)
        nc.scalar.activation(
            sq[:],
            diff[:],
            mybir.ActivationFunctionType.Square,
            scale=1.0 / SCALE,
            accum_out=dist[:],
        )

        out_sb = const_pool.tile([P_LOC, 4], f32)
        nc.vector.tensor_scalar_mul(out_sb[:, 0:2], rm_sb[:, 0:2], dist[:, 0:1])
        nc.vector.tensor_copy(out_sb[:, 2:4], rm_sb[:, 0:2])
        nc.sync.dma_start(out[:], out_sb[:])

    nc.compile()
    return nc


def _pack_chunk(V, off, ksz):
    """V: [NT*N, 600] fp8 (columns = contraction). Returns [ksz, NT*1024]
    with element [k, m*1024 + s*512 + n] = V[m*512+n, off + s*ksz + k]."""
    block = V[:, off : off + 2 * ksz]             # [38400, 2*ksz]
    arr = block.reshape(NT, N, 2, ksz)            # [m, n, s, k]
    return np.ascontiguousarray(arr.transpose(3, 0, 2, 1)).reshape(ksz, NT * 1024)


def kernel(tag_rep, Lp_w, Rp_w, relation, tag1_idx, tag2_idx):
    global LAST_RESULT
    from concourse.bass_utils import run_bass_kernel_spmd

    if "nc" not in _CACHE:
        _CACHE["nc"] = _build_nc()
    nc = _CACHE["nc"]

    tag_rep = np.asarray(tag_rep)
    Lp_w = np.asarray(Lp_w)
    Rp_w = np.asarray(Rp_w)
    rel = np.asarray(relation).astype(np.float32)

    wi = tag_rep[int(tag1_idx)].astype(np.float32)
    wj = tag_rep[int(tag2_idx)].astype(np.float32)
    u = np.concatenate([wi, -wj]).astype(E4)  # [600]
    wd_host = np.zeros((128, 96), dtype=E4)
    for c, (off, ksz) in enumerate(CHUNKS):
        wd_host[0:ksz, 32 * c] = u[off : off + ksz]
        wd_host[0:ksz, 32 * c + 16] = u[off + ksz : off + 2 * ksz]

    # Merged, scaled, quantized stream: [P, H, 600] fp8.
    Q = np.empty((P_TOTAL, H, ROW), dtype=E4)
    Q[:, :, :E] = Lp_w * SCALE
    Q[:, :, E:] = Rp_w * SCALE

    in_maps = []
    for ci in range(N_CORES):
        sl = slice(ci * P_LOC, (ci + 1) * P_LOC)
        V = Q[sl].reshape(P_LOC * H, ROW)  # rows = (p,h) p-major, cols = contraction
        rel_c = rel[sl]
        in_maps.append(
            {
                "x0": _pack_chunk(V, *CHUNKS[0]),
                "x1": _pack_chunk(V, *CHUNKS[1]),
                "x2": _pack_chunk(V, *CHUNKS[2]),
                "wd": wd_host,
                "rm": np.ascontiguousarray(np.stack([rel_c, 1.0 - rel_c], axis=1)),
            }
        )

    kw = {}
    if TRACE:
        kw = dict(trace=True, trace_cores=[0])
    res = run_bass_kernel_spmd(nc, in_maps, core_ids=list(range(N_CORES)), **kw)
    LAST_RESULT = res

    out_full = np.empty((4, P_TOTAL), dtype=np.float32)
    for c in range(N_CORES):
        out_full[:, c * P_LOC : (c + 1) * P_LOC] = res.results[c]["out"].T
    return out_full
